# revision 3
# baseline (speedup 1.0000x reference)
"""Expert-parallel MoE block (dispatch -> gate_up GEMM -> SwiGLU -> down GEMM
-> weighted combine) on 8 Trainium2 NeuronCores.

Sharding: one expert per core. The routing (topk_ids) is known on the host, so
token dispatch happens here in numpy: each core receives only the tokens routed
to its expert (padded to a fixed capacity C), plus that expert's weights, laid
out in the exact transposed/tiled form the TensorEngine consumes (so the device
graph contains no transposes and no collectives). The device computes
Y.T = w2.T @ swiglu(w1.T @ x.T) per expert; the host applies the per-token
combine weights and index-adds the per-expert outputs back into [N, H].

All matmuls run as float32r (full fp32 storage; TensorEngine's full-rate fp32
mode) with fp32 PSUM accumulation.
"""

import os

os.environ.setdefault("JAX_COMPILATION_CACHE_DIR", "/tmp/jax_comp_cache")

import numpy as np

NUM_TOKENS = 4096
HIDDEN = 2048
INTER = 1408
NUM_EXPERTS = 8
NCORES = 8

P = 128
MH = HIDDEN // P  # 16 k-tiles / m-tiles over hidden
MI = INTER // P  # 11 k-tiles / m-tiles over intermediate
M2I = 2 * INTER // P  # 22 m-tiles over gate+up
NFREE = 512  # moving-dim tile (fp32 PSUM bank limit)

_GRAPH_CACHE: dict[int, object] = {}

LAST_EXEC_NS = None
LAST_RESULTS = None


def _build_graph(C: int):
    import concourse.bacc as bacc
    import concourse.mybir as mybir
    import concourse.tile as tile

    F32 = mybir.dt.float32
    F32R = mybir.dt.float32r
    ACT = mybir.ActivationFunctionType
    NT = C // NFREE

    nc = bacc.Bacc()
    xt_ext = nc.declare_dram_parameter("xt", [MH, P, C], F32, isOutput=False)
    w1_ext = nc.declare_dram_parameter("w1p", [M2I, P, HIDDEN], F32, isOutput=False)
    w2_ext = nc.declare_dram_parameter("w2p", [MH, P, INTER], F32, isOutput=False)
    out_ext = nc.declare_dram_parameter("out", [MH, P, C], F32, isOutput=True)

    with tile.TileContext(nc) as tc:
        with (
            tc.tile_pool(name="xpool", bufs=1) as xpool,
            tc.tile_pool(name="apool", bufs=1) as apool,
            tc.tile_pool(name="w1pool", bufs=4) as w1pool,
            tc.tile_pool(name="w2pool", bufs=3) as w2pool,
            tc.tile_pool(name="spool", bufs=4) as spool,
            tc.tile_pool(name="ypool", bufs=3) as ypool,
            tc.tile_pool(name="psum", bufs=8, space="PSUM") as psum,
        ):
            # Resident transposed activations: xbig[:, k, :] = x.T[128k:128(k+1), :]
            xbig = xpool.tile([P, MH, C], F32R)
            for k in range(MH):
                nc.sync.dma_start(out=xbig[:, k, :], in_=xt_ext[k].bitcast(F32R))

            # SwiGLU outputs A.T, keyed by intermediate k-tile
            abig = apool.tile([P, MI, C], F32R)

            # Phase 1: G.T/U.T = w1.T-blocks @ x.T, then A.T = silu(G.T)*U.T
            for m in range(MI):
                wg = w1pool.tile([P, HIDDEN], F32R, tag="w1")
                nc.sync.dma_start(out=wg[:, :], in_=w1_ext[m].bitcast(F32R))
                wu = w1pool.tile([P, HIDDEN], F32R, tag="w1")
                nc.sync.dma_start(out=wu[:, :], in_=w1_ext[m + MI].bitcast(F32R))
                for n in range(NT):
                    ns = slice(n * NFREE, (n + 1) * NFREE)
                    psG = psum.tile([P, NFREE], F32, tag="ps")
                    for k in range(MH):
                        nc.tensor.matmul(
                            psG[:, :],
                            wg[:, k * P : (k + 1) * P],
                            xbig[:, k, ns],
                            start=(k == 0),
                            stop=(k == MH - 1),
                        )
                    psU = psum.tile([P, NFREE], F32, tag="ps")
                    for k in range(MH):
                        nc.tensor.matmul(
                            psU[:, :],
                            wu[:, k * P : (k + 1) * P],
                            xbig[:, k, ns],
                            start=(k == 0),
                            stop=(k == MH - 1),
                        )
                    sil = spool.tile([P, NFREE], F32)
                    nc.scalar.activation(sil[:, :], psG[:, :], ACT.Silu)
                    nc.vector.tensor_mul(abig[:, m, ns], sil[:, :], psU[:, :])

            # Phase 2: Y.T = w2.T-blocks @ A.T
            for mh in range(MH):
                w2t = w2pool.tile([P, INTER], F32R, tag="w2")
                nc.sync.dma_start(out=w2t[:, :], in_=w2_ext[mh].bitcast(F32R))
                yt = ypool.tile([P, C], F32, tag="y")
                for n in range(NT):
                    ns = slice(n * NFREE, (n + 1) * NFREE)
                    psY = psum.tile([P, NFREE], F32, tag="ps")
                    for k in range(MI):
                        nc.tensor.matmul(
                            psY[:, :],
                            w2t[:, k * P : (k + 1) * P],
                            abig[:, k, ns],
                            start=(k == 0),
                            stop=(k == MI - 1),
                        )
                    nc.vector.tensor_copy(yt[:, ns], psY[:, :])
                nc.sync.dma_start(out=out_ext[mh], in_=yt[:, :])

    nc.finalize()
    return nc


def _get_graph(C: int):
    if C not in _GRAPH_CACHE:
        _GRAPH_CACHE[C] = _build_graph(C)
    return _GRAPH_CACHE[C]


def prepare(hidden_states, w1, w2, topk_weights, topk_ids):
    """Host-side dispatch: returns (nc, in_maps, idx, combine, C)."""
    hidden_states = np.asarray(hidden_states, dtype=np.float32)
    w1 = np.asarray(w1, dtype=np.float32)
    w2 = np.asarray(w2, dtype=np.float32)
    topk_weights = np.asarray(topk_weights, dtype=np.float32)
    topk_ids = np.asarray(topk_ids)

    n_tok = hidden_states.shape[0]

    # Per-(token, expert) combine weight, summing duplicate expert hits.
    combine = np.zeros((n_tok, NUM_EXPERTS), np.float32)
    rows = np.arange(n_tok)
    for j in range(topk_ids.shape[1]):
        np.add.at(combine, (rows, topk_ids[:, j]), topk_weights[:, j])

    idx = [np.nonzero(combine[:, e])[0] for e in range(NUM_EXPERTS)]
    max_load = max((len(i) for i in idx), default=1)
    C = max(NFREE, -(-max_load // NFREE) * NFREE)

    in_maps = []
    for e in range(NUM_EXPERTS):
        ids_e = idx[e]
        xT = np.zeros((HIDDEN, C), np.float32)
        if len(ids_e):
            xT[:, : len(ids_e)] = hidden_states[ids_e].T
        # lhsT block layout: w1p[m, p, k*128+c] = w1[e][m*128+c, k*128+p]
        w1p = np.ascontiguousarray(
            w1[e].reshape(M2I, P, MH, P).transpose(0, 3, 2, 1)
        ).reshape(M2I, P, HIDDEN)
        w2p = np.ascontiguousarray(
            w2[e].reshape(MH, P, MI, P).transpose(0, 3, 2, 1)
        ).reshape(MH, P, INTER)
        in_maps.append(
            {
                "xt": np.ascontiguousarray(xT.reshape(MH, P, C)),
                "w1p": w1p,
                "w2p": w2p,
            }
        )
    return _get_graph(C), in_maps, idx, combine, C


def kernel(hidden_states, w1, w2, topk_weights, topk_ids):
    global LAST_EXEC_NS, LAST_RESULTS
    from concourse.bass_utils import run_bass_kernel_spmd

    hidden_states = np.asarray(hidden_states, dtype=np.float32)
    nc, in_maps, idx, combine, C = prepare(
        hidden_states, w1, w2, topk_weights, topk_ids
    )
    n_tok = hidden_states.shape[0]

    trace = os.environ.get("MOE_PROFILE", "0") == "1"
    res = run_bass_kernel_spmd(nc, in_maps, list(range(NCORES)), trace=trace)
    LAST_EXEC_NS = res.exec_time_ns
    LAST_RESULTS = res

    out = np.zeros((n_tok, HIDDEN), np.float32)
    for e in range(NUM_EXPERTS):
        ids_e = idx[e]
        if not len(ids_e):
            continue
        yT = res.results[e]["out"].reshape(HIDDEN, C)
        out[ids_e] += combine[ids_e, e][:, None] * yT[:, : len(ids_e)].T
    return out


# revision 8
# speedup vs baseline: 1.0574x; 1.0574x over previous
"""Expert-parallel MoE block (dispatch -> gate_up GEMM -> SwiGLU -> down GEMM
-> weighted combine) on 8 Trainium2 NeuronCores.

Sharding: one expert per core. The routing (topk_ids) is known on the host, so
token dispatch happens here in numpy: each core receives only the tokens routed
to its expert (padded to a fixed capacity C), plus that expert's weights, laid
out in the exact transposed/tiled form the TensorEngine consumes (so the device
graph contains no transposes and no collectives). The device computes
Y.T = w2.T @ swiglu(w1.T @ x.T) per expert; the host applies the per-token
combine weights and index-adds the per-expert outputs back into [N, H].

All matmuls run as float32r (full fp32 storage; TensorEngine's full-rate fp32
mode) with fp32 PSUM accumulation.
"""

import os

os.environ.setdefault("JAX_COMPILATION_CACHE_DIR", "/tmp/jax_comp_cache")

import numpy as np

NUM_TOKENS = 4096
HIDDEN = 2048
INTER = 1408
NUM_EXPERTS = 8
NCORES = 8

P = 128
MH = HIDDEN // P  # 16 k-tiles / m-tiles over hidden
MI = INTER // P  # 11 k-tiles / m-tiles over intermediate
M2I = 2 * INTER // P  # 22 m-tiles over gate+up
NFREE = 512  # moving-dim tile (fp32 PSUM bank limit)

_GRAPH_CACHE: dict[int, object] = {}

LAST_EXEC_NS = None
LAST_RESULTS = None


def _build_graph(C: int):
    import concourse.bacc as bacc
    import concourse.mybir as mybir
    import concourse.tile as tile

    F32 = mybir.dt.float32
    F32R = mybir.dt.float32r
    ACT = mybir.ActivationFunctionType
    # moving-dim chunks: 512s plus one remainder >=256 (fp32r full-rate floor)
    chunks = [NFREE] * (C // NFREE)
    if C % NFREE:
        chunks.append(C % NFREE)
    offs = [sum(chunks[:i]) for i in range(len(chunks))]
    NT = len(chunks)

    nc = bacc.Bacc()
    xt_ext = nc.declare_dram_parameter("xt", [MH, P, C], F32, isOutput=False)
    w1_ext = nc.declare_dram_parameter("w1p", [M2I, P, HIDDEN], F32, isOutput=False)
    w2_ext = nc.declare_dram_parameter("w2p", [MH, P, INTER], F32, isOutput=False)
    out_ext = nc.declare_dram_parameter("out", [MH, P, C], F32, isOutput=True)

    with tile.TileContext(nc) as tc:
        with (
            tc.tile_pool(name="xpool", bufs=1) as xpool,
            tc.tile_pool(name="apool", bufs=1) as apool,
            tc.tile_pool(name="w1pool", bufs=4) as w1pool,
            tc.tile_pool(name="w2pool", bufs=3) as w2pool,
            tc.tile_pool(name="spool", bufs=4) as spool,
            tc.tile_pool(name="ypool", bufs=3) as ypool,
            tc.tile_pool(name="psum", bufs=8, space="PSUM") as psum,
        ):
            def load_w1(m):
                t = w1pool.tile([P, HIDDEN], F32R, tag="w1")
                nc.sync.dma_start(out=t[:, :], in_=w1_ext[m].bitcast(F32R))
                return t

            # First iteration's weights ahead of the bulk x load so the PE can
            # start as soon as x k-tile 0 lands (DMA rings are FIFO).
            wg0 = load_w1(0)
            wu0 = load_w1(MI)

            # Resident transposed activations: xbig[:, k, :] = x.T[128k:128(k+1), :]
            xbig = xpool.tile([P, MH, C], F32R)
            for k in range(MH):
                nc.sync.dma_start(out=xbig[:, k, :], in_=xt_ext[k].bitcast(F32R))

            # SwiGLU outputs A.T, keyed by intermediate k-tile
            abig = apool.tile([P, MI, C], F32R)

            # Phase 1: G.T/U.T = w1.T-blocks @ x.T, then A.T = silu(G.T)*U.T
            for m in range(MI):
                wg = wg0 if m == 0 else load_w1(m)
                wu = wu0 if m == 0 else load_w1(m + MI)
                for n in range(NT):
                    ns = slice(offs[n], offs[n] + chunks[n])
                    w = chunks[n]
                    psG = psum.tile([P, NFREE], F32, tag="ps")
                    for k in range(MH):
                        nc.tensor.matmul(
                            psG[:, :w],
                            wg[:, k * P : (k + 1) * P],
                            xbig[:, k, ns],
                            start=(k == 0),
                            stop=(k == MH - 1),
                        )
                    psU = psum.tile([P, NFREE], F32, tag="ps")
                    for k in range(MH):
                        nc.tensor.matmul(
                            psU[:, :w],
                            wu[:, k * P : (k + 1) * P],
                            xbig[:, k, ns],
                            start=(k == 0),
                            stop=(k == MH - 1),
                        )
                    sil = spool.tile([P, NFREE], F32)
                    nc.scalar.activation(sil[:, :w], psG[:, :w], ACT.Silu)
                    nc.vector.tensor_mul(abig[:, m, ns], sil[:, :w], psU[:, :w])

            # Phase 2: Y.T = w2.T-blocks @ A.T
            for mh in range(MH):
                w2t = w2pool.tile([P, INTER], F32R, tag="w2")
                nc.sync.dma_start(out=w2t[:, :], in_=w2_ext[mh].bitcast(F32R))
                yt = ypool.tile([P, C], F32, tag="y")
                for n in range(NT):
                    ns = slice(offs[n], offs[n] + chunks[n])
                    w = chunks[n]
                    psY = psum.tile([P, NFREE], F32, tag="ps")
                    for k in range(MI):
                        nc.tensor.matmul(
                            psY[:, :w],
                            w2t[:, k * P : (k + 1) * P],
                            abig[:, k, ns],
                            start=(k == 0),
                            stop=(k == MI - 1),
                        )
                    nc.vector.tensor_copy(yt[:, ns], psY[:, :w])
                nc.sync.dma_start(out=out_ext[mh], in_=yt[:, :])

    nc.finalize()
    return nc


def _get_graph(C: int):
    if C not in _GRAPH_CACHE:
        _GRAPH_CACHE[C] = _build_graph(C)
    return _GRAPH_CACHE[C]


def prepare(hidden_states, w1, w2, topk_weights, topk_ids):
    """Host-side dispatch: returns (nc, in_maps, idx, combine, C)."""
    hidden_states = np.asarray(hidden_states, dtype=np.float32)
    w1 = np.asarray(w1, dtype=np.float32)
    w2 = np.asarray(w2, dtype=np.float32)
    topk_weights = np.asarray(topk_weights, dtype=np.float32)
    topk_ids = np.asarray(topk_ids)

    n_tok = hidden_states.shape[0]

    # Per-(token, expert) combine weight, summing duplicate expert hits.
    combine = np.zeros((n_tok, NUM_EXPERTS), np.float32)
    rows = np.arange(n_tok)
    for j in range(topk_ids.shape[1]):
        np.add.at(combine, (rows, topk_ids[:, j]), topk_weights[:, j])

    idx = [np.nonzero(combine[:, e])[0] for e in range(NUM_EXPERTS)]
    max_load = max((len(i) for i in idx), default=1)
    # capacity = max expert load, padded so every 512-chunk remainder is
    # either 0 or >=256 (fp32r matmuls run at 1/4 rate below 256 free)
    C = max(max_load, 256)
    r = C % NFREE
    if 0 < r < 256:
        C += 256 - r

    in_maps = []
    for e in range(NUM_EXPERTS):
        ids_e = idx[e]
        xT = np.zeros((HIDDEN, C), np.float32)
        if len(ids_e):
            xT[:, : len(ids_e)] = hidden_states[ids_e].T
        # lhsT block layout: w1p[m, p, k*128+c] = w1[e][m*128+c, k*128+p]
        w1p = np.ascontiguousarray(
            w1[e].reshape(M2I, P, MH, P).transpose(0, 3, 2, 1)
        ).reshape(M2I, P, HIDDEN)
        w2p = np.ascontiguousarray(
            w2[e].reshape(MH, P, MI, P).transpose(0, 3, 2, 1)
        ).reshape(MH, P, INTER)
        in_maps.append(
            {
                "xt": np.ascontiguousarray(xT.reshape(MH, P, C)),
                "w1p": w1p,
                "w2p": w2p,
            }
        )
    return _get_graph(C), in_maps, idx, combine, C


def kernel(hidden_states, w1, w2, topk_weights, topk_ids):
    global LAST_EXEC_NS, LAST_RESULTS
    from concourse.bass_utils import run_bass_kernel_spmd

    hidden_states = np.asarray(hidden_states, dtype=np.float32)
    nc, in_maps, idx, combine, C = prepare(
        hidden_states, w1, w2, topk_weights, topk_ids
    )
    n_tok = hidden_states.shape[0]

    trace = os.environ.get("MOE_PROFILE", "0") == "1"
    res = run_bass_kernel_spmd(nc, in_maps, list(range(NCORES)), trace=trace)
    LAST_EXEC_NS = res.exec_time_ns
    LAST_RESULTS = res

    out = np.zeros((n_tok, HIDDEN), np.float32)
    for e in range(NUM_EXPERTS):
        ids_e = idx[e]
        if not len(ids_e):
            continue
        yT = res.results[e]["out"].reshape(HIDDEN, C)
        out[ids_e] += combine[ids_e, e][:, None] * yT[:, : len(ids_e)].T
    return out


# revision 13
# speedup vs baseline: 1.0593x; 1.0018x over previous
"""Expert-parallel MoE block (dispatch -> gate_up GEMM -> SwiGLU -> down GEMM
-> weighted combine) on 8 Trainium2 NeuronCores.

Sharding: one expert per core. The routing (topk_ids) is known on the host, so
token dispatch happens here in numpy: each core receives only the tokens routed
to its expert (padded to a fixed capacity C), plus that expert's weights, laid
out in the exact transposed/tiled form the TensorEngine consumes (so the device
graph contains no transposes and no collectives). The device computes
Y.T = w2.T @ swiglu(w1.T @ x.T) per expert; the host applies the per-token
combine weights and index-adds the per-expert outputs back into [N, H].

All matmuls run as float32r (full fp32 storage; TensorEngine's full-rate fp32
mode) with fp32 PSUM accumulation.
"""

import os

os.environ.setdefault("JAX_COMPILATION_CACHE_DIR", "/tmp/jax_comp_cache")

import numpy as np

NUM_TOKENS = 4096
HIDDEN = 2048
INTER = 1408
NUM_EXPERTS = 8
NCORES = 8

P = 128
MH = HIDDEN // P  # 16 k-tiles / m-tiles over hidden
MI = INTER // P  # 11 k-tiles / m-tiles over intermediate
M2I = 2 * INTER // P  # 22 m-tiles over gate+up
NFREE = 512  # moving-dim tile (fp32 PSUM bank limit)

_GRAPH_CACHE: dict[int, object] = {}

LAST_EXEC_NS = None
LAST_RESULTS = None


def _build_graph(C: int):
    import concourse.bacc as bacc
    import concourse.mybir as mybir
    import concourse.tile as tile

    F32 = mybir.dt.float32
    F32R = mybir.dt.float32r
    ACT = mybir.ActivationFunctionType
    # moving-dim chunks: 512s plus one remainder >=256 (fp32r full-rate floor)
    chunks = [NFREE] * (C // NFREE)
    if C % NFREE:
        chunks.append(C % NFREE)
    offs = [sum(chunks[:i]) for i in range(len(chunks))]
    NT = len(chunks)

    nc = bacc.Bacc()
    xt_ext = nc.declare_dram_parameter("xt", [MH, P, C], F32, isOutput=False)
    w1_ext = nc.declare_dram_parameter("w1p", [M2I, P, HIDDEN], F32, isOutput=False)
    w2_ext = nc.declare_dram_parameter("w2p", [MH, P, INTER], F32, isOutput=False)
    out_ext = nc.declare_dram_parameter("out", [MH, P, C], F32, isOutput=True)

    with tile.TileContext(nc) as tc:
        with (
            tc.tile_pool(name="xpool", bufs=1) as xpool,
            tc.tile_pool(name="apool", bufs=1) as apool,
            tc.tile_pool(name="w1pool", bufs=4) as w1pool,
            tc.tile_pool(name="w2pool", bufs=3) as w2pool,
            tc.tile_pool(name="spool", bufs=4) as spool,
            tc.tile_pool(name="ypool", bufs=3) as ypool,
            tc.tile_pool(name="psum", bufs=8, space="PSUM") as psum,
        ):
            def load_w1(m):
                t = w1pool.tile([P, HIDDEN], F32R, tag="w1")
                nc.sync.dma_start(out=t[:, :], in_=w1_ext[m].bitcast(F32R))
                return t

            # First iteration's gate weights ahead of the bulk x load so the
            # PE can start as soon as x k-tile 0 lands (DMA rings are FIFO).
            wg0 = load_w1(0)

            # Resident transposed activations: xbig[:, k, :] = x.T[128k:128(k+1), :]
            xbig = xpool.tile([P, MH, C], F32R)
            nc.sync.dma_start(out=xbig[:, 0, :], in_=xt_ext[0].bitcast(F32R))
            wu0 = load_w1(MI)
            for k in range(1, MH):
                nc.sync.dma_start(out=xbig[:, k, :], in_=xt_ext[k].bitcast(F32R))

            # SwiGLU outputs A.T, keyed by intermediate k-tile
            abig = apool.tile([P, MI, C], F32R)

            # Phase 1: G.T/U.T = w1.T-blocks @ x.T, then A.T = silu(G.T)*U.T
            # NOTE: accumulation groups must stay contiguous per PSUM bank —
            # interleaving groups across banks crashes the exec unit.
            for m in range(MI):
                wg = wg0 if m == 0 else load_w1(m)
                wu = wu0 if m == 0 else load_w1(m + MI)
                for n in range(NT):
                    ns = slice(offs[n], offs[n] + chunks[n])
                    w = chunks[n]
                    psG = psum.tile([P, NFREE], F32, tag="ps", name=f"psG{m}_{n}")
                    for k in range(MH):
                        nc.tensor.matmul(
                            psG[:, :w],
                            wg[:, k * P : (k + 1) * P],
                            xbig[:, k, ns],
                            start=(k == 0),
                            stop=(k == MH - 1),
                        )
                    psU = psum.tile([P, NFREE], F32, tag="ps", name=f"psU{m}_{n}")
                    for k in range(MH):
                        nc.tensor.matmul(
                            psU[:, :w],
                            wu[:, k * P : (k + 1) * P],
                            xbig[:, k, ns],
                            start=(k == 0),
                            stop=(k == MH - 1),
                        )
                    sil = spool.tile([P, NFREE], F32)
                    nc.scalar.activation(sil[:, :w], psG[:, :w], ACT.Silu)
                    nc.vector.tensor_mul(abig[:, m, ns], sil[:, :w], psU[:, :w])

            # Phase 2: Y.T = w2.T-blocks @ A.T
            for mh in range(MH):
                w2t = w2pool.tile([P, INTER], F32R, tag="w2")
                nc.sync.dma_start(out=w2t[:, :], in_=w2_ext[mh].bitcast(F32R))
                yt = ypool.tile([P, C], F32, tag="y")
                for n in range(NT):
                    ns = slice(offs[n], offs[n] + chunks[n])
                    w = chunks[n]
                    psY = psum.tile([P, NFREE], F32, tag="ps", name=f"psY{mh}_{n}")
                    for k in range(MI):
                        nc.tensor.matmul(
                            psY[:, :w],
                            w2t[:, k * P : (k + 1) * P],
                            abig[:, k, ns],
                            start=(k == 0),
                            stop=(k == MI - 1),
                        )
                    nc.vector.tensor_copy(yt[:, ns], psY[:, :w])
                nc.sync.dma_start(out=out_ext[mh], in_=yt[:, :])

    nc.finalize()
    return nc


def _get_graph(C: int):
    if C not in _GRAPH_CACHE:
        _GRAPH_CACHE[C] = _build_graph(C)
    return _GRAPH_CACHE[C]


def prepare(hidden_states, w1, w2, topk_weights, topk_ids):
    """Host-side dispatch: returns (nc, in_maps, idx, combine, C)."""
    hidden_states = np.asarray(hidden_states, dtype=np.float32)
    w1 = np.asarray(w1, dtype=np.float32)
    w2 = np.asarray(w2, dtype=np.float32)
    topk_weights = np.asarray(topk_weights, dtype=np.float32)
    topk_ids = np.asarray(topk_ids)

    n_tok = hidden_states.shape[0]

    # Per-(token, expert) combine weight, summing duplicate expert hits.
    combine = np.zeros((n_tok, NUM_EXPERTS), np.float32)
    rows = np.arange(n_tok)
    for j in range(topk_ids.shape[1]):
        np.add.at(combine, (rows, topk_ids[:, j]), topk_weights[:, j])

    idx = [np.nonzero(combine[:, e])[0] for e in range(NUM_EXPERTS)]
    max_load = max((len(i) for i in idx), default=1)
    # capacity = max expert load, padded so every 512-chunk remainder is
    # either 0 or >=256 (fp32r matmuls run at 1/4 rate below 256 free)
    C = max(max_load, 256)
    r = C % NFREE
    if 0 < r < 256:
        C += 256 - r

    in_maps = []
    for e in range(NUM_EXPERTS):
        ids_e = idx[e]
        xT = np.zeros((HIDDEN, C), np.float32)
        if len(ids_e):
            xT[:, : len(ids_e)] = hidden_states[ids_e].T
        # lhsT block layout: w1p[m, p, k*128+c] = w1[e][m*128+c, k*128+p]
        w1p = np.ascontiguousarray(
            w1[e].reshape(M2I, P, MH, P).transpose(0, 3, 2, 1)
        ).reshape(M2I, P, HIDDEN)
        w2p = np.ascontiguousarray(
            w2[e].reshape(MH, P, MI, P).transpose(0, 3, 2, 1)
        ).reshape(MH, P, INTER)
        in_maps.append(
            {
                "xt": np.ascontiguousarray(xT.reshape(MH, P, C)),
                "w1p": w1p,
                "w2p": w2p,
            }
        )
    return _get_graph(C), in_maps, idx, combine, C


def kernel(hidden_states, w1, w2, topk_weights, topk_ids):
    global LAST_EXEC_NS, LAST_RESULTS
    from concourse.bass_utils import run_bass_kernel_spmd

    hidden_states = np.asarray(hidden_states, dtype=np.float32)
    nc, in_maps, idx, combine, C = prepare(
        hidden_states, w1, w2, topk_weights, topk_ids
    )
    n_tok = hidden_states.shape[0]

    trace = os.environ.get("MOE_PROFILE", "0") == "1"
    res = run_bass_kernel_spmd(nc, in_maps, list(range(NCORES)), trace=trace)
    LAST_EXEC_NS = res.exec_time_ns
    LAST_RESULTS = res

    out = np.zeros((n_tok, HIDDEN), np.float32)
    for e in range(NUM_EXPERTS):
        ids_e = idx[e]
        if not len(ids_e):
            continue
        yT = res.results[e]["out"].reshape(HIDDEN, C)
        out[ids_e] += combine[ids_e, e][:, None] * yT[:, : len(ids_e)].T
    return out
